# revision 28
# baseline (speedup 1.0000x reference)
"""BatchGRU Trainium2 kernel (fp8 DoubleRow version).

Bidirectional GRU over 256 ragged graph sequences (L=128, H=1024),
data-parallel over graphs x direction on 8 NeuronCores:
  cores 0-3: forward direction, 64 graph-ranks each
  cores 4-7: backward direction, 64 graph-ranks each

Numerics: matmuls run as fp8(e4m3) DoubleRow pairs. The hidden state h is
stored transposed as fp8 hi (+ same-scale lo residual) pairs; per-gate the
recurrent stream is either
  "h8"  : hi chunk-pairs (x) single-fp8 W         (4 pair-mms / 512 cols)
  "hres": (hi,lo) per-chunk pairs (x) W duplicated (8 pair-mms / 512 cols)
The projection uses single-fp8 msgs (x) hi|lo residual W. PSUM carries
scale SH*SW; activations apply 1/(SH*SW) on ACT. Gate tensors are bf16.
"""

import numpy as np
import ml_dtypes
_bf16 = ml_dtypes.bfloat16
_e4 = ml_dtypes.float8_e4m3

H = 1024
G = 256
L = 128
N_CORES = 8
GPC = 64  # graph-ranks per core (one direction)
KC = 8  # H // 128 contraction chunks
H3 = 3 * H
SH = 16.0    # state / msgs fp8 scale
SW = 256.0   # weight fp8 scale
SC = SH * SW  # psum scale

# per-gate recurrent stream mode:
#   "h8"  : hi chunk-pairs x W8-tensor cols      (4 pair-mms / 512 cols)
#   "hres": (hi,lo) chunk pairs x Wd-tensor cols (8 pair-mms / 512 cols)
#   "3p"  : hres on Whi (Wd) + h8 on Wlo (W8)    (12 pair-mms / 512 cols)
# For "3p" the W8 tensor carries the lo residual; for "h8" it carries hi.
REC_SCHEME = {"r": "hres", "z": "3p", "n": "3p"}
# projection: include W-lo residual blocks per gate?
PROJ_LO = {"r": True, "z": True, "n": True}

_PROG_CACHE = {}
_LAST_IN_MAPS = None


def _build_program(direction, caps, n_steps):
    import concourse.mybir as mybir
    import concourse.tile as tile
    from concourse import bacc
    from concourse.masks import make_identity

    F32 = mybir.dt.float32
    BF16 = mybir.dt.bfloat16
    F8 = mybir.dt.float8e4
    AF = mybir.ActivationFunctionType
    OP = mybir.AluOpType
    DR = mybir.MatmulPerfMode.DoubleRow

    caps = list(caps)
    base = np.concatenate([[0], np.cumsum(caps)]).astype(int)
    S = int(base[-1])
    SP = ((S + 127) // 128) * 128
    NT = SP // 128
    if direction == "f":
        kreal = [sum(1 for c in caps if c > t) for t in range(n_steps)]
    else:
        kreal = [sum(1 for c in caps if c >= L - t) for t in range(n_steps)]
    shift = [0 if direction == "f" else L - c for c in caps]

    G0 = {"r": 0, "z": 1024, "n": 2048}
    # projection blocks: (w_col in packed 6144, psum col in 3072)
    pblocks = []
    for gate in ("r", "n", "z"):
        for v in range(2 if PROJ_LO[gate] else 1):
            for hf in range(2):
                pblocks.append((v * 3072 + G0[gate] + hf * 512, G0[gate] + hf * 512))

    # recurrence: per gate, mode decides pair structure; W tensors:
    #   w8  cols: gates with mode h8 (hi) or 3p (lo residual)
    #   wd  cols: gates with mode hres/3p (hi, duplicated in pair dim)
    w8_gates = [g for g in ("r", "z", "n") if REC_SCHEME[g] in ("h8", "3p")]
    wd_gates = [g for g in ("r", "z", "n") if REC_SCHEME[g] in ("hres", "3p")]
    w8_off = {g: i * 1024 for i, g in enumerate(w8_gates)}
    wd_off = {g: i * 1024 for i, g in enumerate(wd_gates)}
    W8C = max(1024 * len(w8_gates), 512)
    WDC = max(1024 * len(wd_gates), 512)

    nc = bacc.Bacc("TRN2", target_bir_lowering=False, debug=False)

    msgsT8_d = nc.dram_tensor("msgsT8", [NT, 128, KC, 128], F8,
                              kind="ExternalInput").ap()
    w_ih_d = nc.dram_tensor("w_ih", [KC, 128, 6144], F8, kind="ExternalInput").ap()
    w_hh8_d = nc.dram_tensor("w_hh8", [KC, 128, W8C], F8, kind="ExternalInput").ap()
    w_hhd_d = nc.dram_tensor("w_hhd", [KC, 128, 2, WDC], F8,
                             kind="ExternalInput").ap()
    pbias_d = nc.dram_tensor("pbias", [1, H3], BF16, kind="ExternalInput").ap()
    padgr_d = nc.dram_tensor("padgr", [GPC, H3], BF16, kind="ExternalInput").ap()
    bhh_n_d = nc.dram_tensor("bhh_n", [1, H], BF16, kind="ExternalInput").ap()
    id64_d = nc.dram_tensor("id64", [GPC, GPC], BF16, kind="ExternalInput").ap()
    ones_d = nc.dram_tensor("ones", [1, 128], BF16, kind="ExternalInput").ap()
    h0T8_d = nc.dram_tensor("h0T8", [128, KC, 2, GPC], F8,
                            kind="ExternalInput").ap()
    h0n_d = nc.dram_tensor("h0n", [GPC, H], BF16, kind="ExternalInput").ap()
    out_stage = nc.dram_tensor("out_stage", [L, GPC, H], BF16,
                               kind="ExternalOutput").ap()
    xg_tm = nc.dram_tensor("xg_tm", [L, GPC, H3], BF16).ap()  # scratch

    segs_by_tile = [[] for _ in range(NT)]
    for r in range(GPC):
        q = 0
        while q < caps[r]:
            row = base[r] + q
            ti = row // 128
            take = min(caps[r] - q, (ti + 1) * 128 - row)
            segs_by_tile[ti].append((row - ti * 128, take, r, q))
            q += take

    with tile.TileContext(nc) as tc:
        with (
            tc.tile_pool(name="wpool", bufs=1) as wpool,
            tc.tile_pool(name="consts", bufs=1) as consts,
            tc.tile_pool(name="state", bufs=2) as state,
        ):
            ident = consts.tile([GPC, GPC], BF16)
            make_identity(nc, ident)
            ones_r = consts.tile([1, 128], BF16)
            nc.sync.dma_start(out=ones_r, in_=ones_d)
            pbias_bc = consts.tile([128, H3], BF16)
            nc.sync.dma_start(out=pbias_bc, in_=pbias_d.to_broadcast((128, H3)))
            bhh_n_s = consts.tile([1, H], BF16)
            nc.sync.dma_start(out=bhh_n_s, in_=bhh_n_d)
            id64_s = consts.tile([GPC, GPC], BF16)
            nc.sync.dma_start(out=id64_s, in_=id64_d)

            w_ih_s = wpool.tile([128, KC, 6144], F8, tag="wih")
            for k in range(KC):
                nc.sync.dma_start(out=w_ih_s[:, k, :], in_=w_ih_d[k])
            w_hh8_s = wpool.tile([128, KC, W8C], F8, tag="whh8")
            for k in range(KC):
                nc.sync.dma_start(out=w_hh8_s[:, k, :], in_=w_hh8_d[k])
            w_hhd_s = wpool.tile([128, KC, 2, WDC], F8, tag="whhd")
            for k in range(KC):
                nc.sync.dma_start(out=w_hhd_s[:, k, :, :], in_=w_hhd_d[k])

            # ---------------- Phase A: projection (compact layout) --------
            with (
                tc.tile_pool(name="pa", bufs=1, space="PSUM") as pa,
                tc.tile_pool(name="aw", bufs=4) as aw,
                tc.tile_pool(name="mrp", bufs=3) as mrp,
            ):
                for ti in range(NT):
                    if not segs_by_tile[ti]:
                        continue
                    mr = mrp.tile([128, KC, 128], F8, tag="mr")
                    nc.scalar.dma_start(out=mr, in_=msgsT8_d[ti])
                    pm0 = pa.tile([128, 1536], F32, tag="pm0")
                    pm1 = pa.tile([128, 1536], F32, tag="pm1")
                    pms = (pm0, pm1)
                    xgs = aw.tile([128, H3], BF16, tag="xgs")
                    # run all pm0 blocks first so its copy overlaps pm1's
                    # stream (pa has a single buffer set); exactly one
                    # start=True and one stop=True per psum 512-col region
                    for half in range(2):
                        blks = [b for b in pblocks if b[1] // 1536 == half]
                        seen, last = set(), {}
                        for i, (wc, pc) in enumerate(blks):
                            last[pc] = i
                        for kp in range(4):
                            lhsT = mr[:, 2 * kp:2 * kp + 2, :]
                            for i, (wc, pc) in enumerate(blks):
                                po = pc % 1536
                                st = (kp == 0) and pc not in seen
                                seen.add(pc)
                                nc.tensor.matmul(
                                    pms[half][:, po:po + 512],
                                    lhsT, w_ih_s[:, 2 * kp:2 * kp + 2, wc:wc + 512],
                                    start=st,
                                    stop=(kp == 3 and last[pc] == i),
                                    perf_mode=DR)
                        nc.vector.tensor_tensor(
                            xgs[:, half * 1536:(half + 1) * 1536], pms[half],
                            pbias_bc[:, half * 1536:(half + 1) * 1536], op=OP.add)
                    for si, (r0, nr, rank, q0) in enumerate(segs_by_tile[ti]):
                        t0 = q0 + shift[rank]
                        eng = nc.sync if si % 2 == 0 else nc.scalar
                        eng.dma_start(
                            out=xg_tm[t0:t0 + nr, rank, :],
                            in_=xgs[r0:r0 + nr, :])

            # ---------------- Phase B: recurrence ----------------
            hT = state.tile([128, KC, 2, GPC], F8, tag="hT")
            nc.sync.dma_start(out=hT, in_=h0T8_d)
            h_s = state.tile([GPC, H], BF16, tag="h")
            nc.sync.dma_start(out=h_s, in_=h0n_d)

            with (
                tc.tile_pool(name="pb", bufs=1, space="PSUM") as pb,
                tc.tile_pool(name="ptr", bufs=2, space="PSUM") as ptrp,
                tc.tile_pool(name="gw", bufs=1) as gw,
                tc.tile_pool(name="xn", bufs=3) as xnp,
            ):
                for t in range(n_steps):
                    xg_s = xnp.tile([GPC, H3], BF16, tag="xg")
                    k_t = kreal[t]
                    if k_t > 0:
                        nc.sync.dma_start(out=xg_s[:k_t, :],
                                            in_=xg_tm[t, :k_t, :])
                    if k_t < GPC:
                        nc.sync.dma_start(out=xg_s[k_t:, :],
                                            in_=padgr_d[k_t:, :])

                    pr = pb.tile([GPC, H], F32, tag="pr")
                    pz = pb.tile([GPC, H], F32, tag="pz")
                    pn = pb.tile([GPC, H], F32, tag="pn")
                    ps = {"r": pr, "z": pz, "n": pn}

                    # injections (start=True per bank)
                    for hf in range(2):
                        sl = slice(hf * 512, hf * 512 + 512)
                        nc.tensor.matmul(pr[:, sl], id64_s,
                                         xg_s[:, hf * 512:hf * 512 + 512],
                                         start=True, stop=False)
                        nc.tensor.matmul(pz[:, sl], id64_s,
                                         xg_s[:, 1024 + hf * 512:1024 + hf * 512 + 512],
                                         start=True, stop=False)
                        nc.tensor.matmul(pn[:, sl], ones_r[:, :GPC],
                                         bhh_n_s[:, sl], start=True, stop=False)
                    # weight streams, two chunk-groups (0-3, 4-7), k-outer so
                    # each stationary (hT pair) is loaded once and reused
                    # across all gate blocks (LDWEIGHTS amortization on HW)
                    for grp in range(2):
                        for k in range(4 * grp, 4 * grp + 4):
                            for gate in ("r", "n", "z"):
                                mode = REC_SCHEME[gate]
                                if mode not in ("hres", "3p"):
                                    continue
                                for hf in range(2):
                                    pc = hf * 512
                                    wc = wd_off[gate] + pc
                                    nc.tensor.matmul(
                                        ps[gate][:, pc:pc + 512],
                                        hT[:, k, :, :],
                                        w_hhd_s[:, k, :, wc:wc + 512],
                                        start=False,
                                        stop=(k == 7 and mode == "hres"),
                                        perf_mode=DR)
                        for kp in (2 * grp, 2 * grp + 1):
                            for gate in ("r", "n", "z"):
                                mode = REC_SCHEME[gate]
                                if mode not in ("h8", "3p"):
                                    continue
                                for hf in range(2):
                                    pc = hf * 512
                                    wc = w8_off[gate] + pc
                                    nc.tensor.matmul(
                                        ps[gate][:, pc:pc + 512],
                                        hT[:, 2 * kp:2 * kp + 2, 0, :],
                                        w_hh8_s[:, 2 * kp:2 * kp + 2, wc:wc + 512],
                                        start=False, stop=(kp == 3),
                                        perf_mode=DR)

                    r_g = gw.tile([GPC, H], BF16, tag="r")
                    u = gw.tile([GPC, H], BF16, tag="u")
                    n_g = gw.tile([GPC, H], BF16, tag="n")
                    z_g = gw.tile([GPC, H], BF16, tag="z")
                    d = gw.tile([GPC, H], BF16, tag="d")
                    e = gw.tile([GPC, H], BF16, tag="e")
                    h_new = state.tile([GPC, H], BF16, tag="h")
                    hT_next = state.tile([128, KC, 2, GPC], F8, tag="hT")
                    ptr_t = ptrp.tile([128, KC, GPC], BF16, tag="ptr")

                    for hf in range(2):
                        sl = slice(hf * 512, hf * 512 + 512)
                        nc.scalar.activation(r_g[:, sl], pr[:, sl],
                                             AF.Sigmoid, scale=1.0 / SC)
                        nc.vector.tensor_tensor(u[:, sl], r_g[:, sl],
                                                pn[:, sl], op=OP.mult)
                        nc.vector.tensor_tensor(
                            u[:, sl], u[:, sl],
                            xg_s[:, 2048 + hf * 512:2048 + hf * 512 + 512],
                            op=OP.add)
                        nc.scalar.activation(n_g[:, sl], u[:, sl],
                                             AF.Tanh, scale=1.0 / SC)
                        nc.gpsimd.tensor_sub(d[:, sl], h_s[:, sl], n_g[:, sl])
                        nc.scalar.activation(z_g[:, sl], pz[:, sl],
                                             AF.Sigmoid, scale=1.0 / SC)
                        for q in range(2):
                            qs = slice(hf * 512 + q * 256, hf * 512 + q * 256 + 256)
                            nc.gpsimd.tensor_tensor(e[:, qs], z_g[:, qs],
                                                    d[:, qs], op=OP.mult)
                            nc.gpsimd.tensor_tensor(h_new[:, qs], n_g[:, qs],
                                                    e[:, qs], op=OP.add)
                            if t < n_steps - 1:
                                c0 = 4 * hf + q * 2
                                for c in (c0, c0 + 1):
                                    nc.tensor.transpose(
                                        ptr_t[:, c, :],
                                        h_new[:, c * 128:(c + 1) * 128],
                                        ident)
                        if t < n_steps - 1:
                            hsl = slice(4 * hf, 4 * hf + 4)
                            nc.vector.tensor_scalar_mul(
                                hT_next[:, hsl, 0, :], ptr_t[:, hsl, :], SH)
                            nc.vector.scalar_tensor_tensor(
                                hT_next[:, hsl, 1, :], ptr_t[:, hsl, :], SH,
                                hT_next[:, hsl, 0, :],
                                op0=OP.mult, op1=OP.subtract)

                    nout = max(k_t, 1)
                    nc.scalar.dma_start(out=out_stage[t, :nout, :],
                                        in_=h_new[:nout, :])
                    if t < n_steps - 1:
                        hT = hT_next
                    h_s = h_new

    nc.compile()
    return nc, SP


def _get_programs(caps):
    key = tuple(caps)
    if key not in _PROG_CACHE:
        nf, SPf = _build_program("f", caps, int(caps[0]))
        nb, SPb = _build_program("b", caps, L)
        _PROG_CACHE[key] = (nf, nb, SPf)
    return _PROG_CACHE[key]


def _make_runner(nc, n_group, dev_offset):
    import jax
    from jax.sharding import Mesh, PartitionSpec
    from jax.experimental.shard_map import shard_map
    import concourse.mybir as mybir
    from concourse.bass2jax import (_bass_exec_p, install_neuronx_cc_hook,
                                    partition_id_tensor)

    install_neuronx_cc_hook()
    pname = nc.partition_id_tensor.name if nc.partition_id_tensor else None
    in_names, out_names, out_avals, zero_outs = [], [], [], []
    for alloc in nc.m.functions[0].allocations:
        if not isinstance(alloc, mybir.__dict__["MemoryLocationSet"]):
            continue
        name = alloc.memorylocations[0].name
        if alloc.kind == "ExternalInput":
            if name != pname:
                in_names.append(name)
        elif alloc.kind == "ExternalOutput":
            out_names.append(name)
            shape = tuple(alloc.tensor_shape)
            dtype = mybir.dt.np(alloc.dtype)
            out_avals.append(jax.core.ShapedArray(shape, dtype))
            zero_outs.append(np.zeros(shape, dtype))
    n_params = len(in_names)
    n_outs = len(out_avals)
    all_in = in_names + out_names + ([pname] if pname else [])

    def _body(*args):
        ops = list(args)
        if pname is not None:
            ops.append(partition_id_tensor())
        outs = _bass_exec_p.bind(
            *ops, out_avals=tuple(out_avals), in_names=tuple(all_in),
            out_names=tuple(out_names), lowering_input_output_aliases=(),
            sim_require_finite=True, sim_require_nnan=True, nc=nc)
        return tuple(outs)

    devices = jax.devices()[dev_offset:dev_offset + n_group]
    mesh = Mesh(np.asarray(devices), ("core",))
    jf = jax.jit(shard_map(_body, mesh=mesh,
                           in_specs=(PartitionSpec("core"),) * (n_params + n_outs),
                           out_specs=(PartitionSpec("core"),) * n_outs,
                           check_rep=False), keep_unused=True)
    sh = jax.sharding.NamedSharding(mesh, PartitionSpec("core"))

    def run(in_maps):
        import jax as _jax
        concat = [np.concatenate([np.asarray(in_maps[c][nm]) for c in range(n_group)], axis=0)
                  for nm in in_names]
        concat += [np.concatenate([z] * n_group, axis=0) for z in zero_outs]
        dev_in = [_jax.device_put(a, sh) for a in concat]
        res = jf(*dev_in)
        return res, out_names

    return run


def _q8(x, scale):
    return (np.asarray(x, np.float32) * scale).astype(_e4)


def kernel(h_atom, bias, w_ih_f, w_hh_f, b_ih_f, b_hh_f,
           w_ih_b, w_hh_b, b_ih_b, b_hh_b, batch, num_graphs, pad_len):
    import jax

    h_atom = np.asarray(h_atom, dtype=np.float32)
    batch_np = np.asarray(batch).astype(np.int64)
    n_atoms = h_atom.shape[0]

    counts = np.bincount(batch_np, minlength=G).astype(np.int64)
    start = np.concatenate([[0], np.cumsum(counts)[:-1]])
    pos = np.arange(n_atoms) - start[batch_np]

    order = np.argsort(-counts, kind="stable")
    ranks = [order[4 * np.arange(GPC) + c] for c in range(4)]
    caps = []
    for i in range(GPC):
        m = max(counts[order[4 * i + c]] for c in range(4))
        caps.append(int(max(8, ((m + 7) // 8) * 8)))
    caps = [min(c, L) for c in caps]

    nf, nb, SP = _get_programs(caps)
    base = np.concatenate([[0], np.cumsum(caps)]).astype(int)
    NT = SP // 128

    msg_all = np.maximum(h_atom + np.asarray(bias, np.float32), 0.0)

    w8_gates = [g for g in ("r", "z", "n") if REC_SCHEME[g] in ("h8", "3p")]
    wd_gates = [g for g in ("r", "z", "n") if REC_SCHEME[g] in ("hres", "3p")]
    W8C = max(1024 * len(w8_gates), 512)
    WDC = max(1024 * len(wd_gates), 512)
    GSL = {"r": slice(0, 1024), "z": slice(1024, 2048), "n": slice(2048, 3072)}

    def pack_w_ih(w):
        wT = np.ascontiguousarray(np.asarray(w, np.float32).T)  # [H, 3H]
        hi = _q8(wT, SW)
        lo = (wT * SW - hi.astype(np.float32)).astype(_e4)
        packed = np.concatenate(
            [hi.astype(np.float32), lo.astype(np.float32)], axis=1)
        return packed.astype(_e4).reshape(KC, 128, 6144)

    def pack_w_hh(w):
        wT = np.ascontiguousarray(np.asarray(w, np.float32).T)  # [H, 3H]
        hi = _q8(wT, SW)  # fp8 of W*SW
        lo = (wT * SW - hi.astype(np.float32)).astype(_e4)
        w8 = np.zeros((H, W8C), _e4)
        for i, g in enumerate(w8_gates):
            src = lo if REC_SCHEME[g] == "3p" else hi
            w8[:, i * 1024:(i + 1) * 1024] = src[:, GSL[g]]
        wd = np.zeros((H, WDC), _e4)
        for i, g in enumerate(wd_gates):
            wd[:, i * 1024:(i + 1) * 1024] = hi[:, GSL[g]]
        w8q = w8.reshape(KC, 128, W8C)
        wdq = wd.reshape(KC, 128, WDC)
        wdd = np.stack([wdq, wdq], axis=2)  # [KC, 128, 2, WDC]
        return w8q, wdd

    def prep_core(core):
        d = "f" if core < 4 else "b"
        glist = ranks[core % 4]
        w_ih = w_ih_f if d == "f" else w_ih_b
        w_hh = w_hh_f if d == "f" else w_hh_b
        b_ih = np.asarray(b_ih_f if d == "f" else b_ih_b, dtype=np.float32)
        b_hh = np.asarray(b_hh_f if d == "f" else b_hh_b, dtype=np.float32)

        hc = np.zeros((SP, H), dtype=np.float32)   # compact msgs
        h0 = np.zeros((GPC, H), dtype=np.float32)
        for i, g in enumerate(glist):
            cnt = int(counts[g])
            rows = msg_all[start[g]:start[g] + min(cnt, caps[i])]
            if d == "b":
                rows = rows[::-1]
                off = caps[i] - len(rows)
            else:
                off = 0
            hc[base[i] + off:base[i] + off + len(rows)] = rows
            if cnt > 0:
                h0[i] = h_atom[start[g]:start[g] + cnt].max(axis=0)

        msgsT8 = _q8(hc, SH).reshape(NT, 128, KC, 128).transpose(0, 3, 2, 1)
        h0T = h0.T.reshape(KC, 128, GPC).transpose(1, 0, 2)  # [128, KC, GPC]
        h0hi = _q8(h0T, SH)
        h0lo = (h0T * SH - h0hi.astype(np.float32)).astype(_e4)
        h0T8 = np.stack([h0hi, h0lo], axis=2)  # [128, KC, 2, GPC]

        w8q, wdd = pack_w_hh(w_hh)

        pb = np.empty((1, H3), dtype=np.float32)
        pb[0, :2048] = b_ih[:2048] + b_hh[:2048]
        pb[0, 2048:] = b_ih[2048:]
        pb *= SC
        return {
            "msgsT8": np.ascontiguousarray(msgsT8),
            "w_ih": pack_w_ih(w_ih),
            "w_hh8": w8q,
            "w_hhd": wdd,
            "pbias": pb.astype(_bf16),
            "padgr": np.tile(pb, (GPC, 1)).astype(_bf16),
            "bhh_n": (b_hh[2048:].reshape(1, H) * SC).astype(_bf16),
            "id64": np.eye(GPC, dtype=np.float32).astype(_bf16),
            "ones": np.ones((1, 128), dtype=np.float32).astype(_bf16),
            "h0T8": h0T8,
            "h0n": h0.astype(_bf16),
        }

    in_maps = [prep_core(c) for c in range(N_CORES)]
    global _LAST_IN_MAPS
    _LAST_IN_MAPS = in_maps

    rkey = ("runners",) + tuple(caps)
    if rkey not in _PROG_CACHE:
        _PROG_CACHE[rkey] = (_make_runner(nf, 4, 0), _make_runner(nb, 4, 4))
    run_f, run_b = _PROG_CACHE[rkey]

    res_f, names_f = run_f(in_maps[:4])
    res_b, names_b = run_b(in_maps[4:])
    jax.block_until_ready(res_f)
    jax.block_until_ready(res_b)

    stage_f = np.asarray(res_f[names_f.index("out_stage")]).astype(np.float32)
    stage_b = np.asarray(res_b[names_b.index("out_stage")]).astype(np.float32)

    out = np.empty((n_atoms, 2 * H), dtype=np.float32)
    for core in range(4):
        glist = ranks[core]
        inv = np.empty(G, dtype=np.int64)
        inv[glist] = np.arange(GPC)
        sel = np.isin(batch_np, glist)
        gi = inv[batch_np[sel]]
        p = np.minimum(pos[sel], L - 1)
        out[sel, :H] = stage_f[core * L + p, gi]
        out[sel, H:] = stage_b[core * L + (L - 1 - p), gi]
    return out


# revision 29
# speedup vs baseline: 1.3182x; 1.3182x over previous
"""BatchGRU Trainium2 kernel.

Bidirectional GRU over 256 ragged graph sequences (L=128, H=1024),
data-parallel over graphs x direction on 8 NeuronCores:
  cores 0-3: forward direction, 64 graph-ranks each
  cores 4-7: backward direction, 64 graph-ranks each

Two programs (one per direction group) run concurrently on disjoint core
groups. Graphs are sorted by atom count and dealt 4-way so all cores share
one descending per-rank capacity profile (caps, multiples of 8) -> every
DMA access pattern is identical across the cores of a group (SPMD).

Per core:
  A) projection over the COMPACT slot layout (sum(caps) rows, not 64*128):
     PE-transpose atoms -> h^T chunks; h0 via free-dim reduce_max (pads are
     -1e30); msgs^T = Relu(h^T + bias) fused on ACT; matmul msgs^T @ w_ih^T
     with biases folded in as a rank-1 ones-row matmul; xg stored time-major
     per rank (bwd stores end-aligned: row q -> step q + 128 - cap_r).
  B) recurrence (fwd: caps[0] steps, bwd: 128): per step one prefix DMA of
     real xg rows + one const DMA for padded ranks; hg = h @ w_hh^T on PE
     (fp32r, W as moving operand; xg and b_hh_n injected into PSUM via
     identity / ones rank-1 matmuls), gates on ACT/DVE/GPSIMD, 8 small PE
     transposes produce the next stationary h^T; h stored time-major.
"""

import numpy as np
import ml_dtypes
_bf16 = ml_dtypes.bfloat16

H = 1024
G = 256
L = 128
N_CORES = 8
GPC = 64  # graph-ranks per core (one direction)
KC = 8  # H // 128 contraction chunks
H3 = 3 * H
PAD_VAL = -1e30

_PROG_CACHE = {}
_LAST_IN_MAPS = None


def _build_program(direction, caps, n_steps, dsub='dve', xnbufs=3, gwbufs=1, htsplit=False):
    import concourse.mybir as mybir
    import concourse.tile as tile
    from concourse import bacc
    from concourse.masks import make_identity

    F32 = mybir.dt.float32
    F32R = mybir.dt.float32r
    BF16 = mybir.dt.bfloat16
    AF = mybir.ActivationFunctionType
    OP = mybir.AluOpType
    AX = mybir.AxisListType

    caps = list(caps)
    base = np.concatenate([[0], np.cumsum(caps)]).astype(int)
    S = int(base[-1])
    SP = ((S + 127) // 128) * 128
    NT = SP // 128
    # per-step prefix of ranks whose xg row at that step is real (stored)
    if direction == "f":
        kreal = [sum(1 for c in caps if c > t) for t in range(n_steps)]
    else:
        kreal = [sum(1 for c in caps if c >= L - t) for t in range(n_steps)]
    shift = [0 if direction == "f" else L - c for c in caps]

    nc = bacc.Bacc("TRN2", target_bir_lowering=False, debug=False)

    h_cmp = nc.dram_tensor("h_cmp", [SP, H], F32, kind="ExternalInput").ap()
    w_ihT = nc.dram_tensor("w_ihT", [KC, 128, H3], BF16, kind="ExternalInput").ap()
    w_hhT = nc.dram_tensor("w_hhT", [KC, 128, H3], BF16, kind="ExternalInput").ap()
    biasT = nc.dram_tensor("biasT", [128, KC], F32, kind="ExternalInput").ap()
    pbias = nc.dram_tensor("pbias", [1, H3], F32R, kind="ExternalInput").ap()
    padgr = nc.dram_tensor("padgr", [GPC, H3], F32R, kind="ExternalInput").ap()
    bhh_n = nc.dram_tensor("bhh_n", [1, H], F32R, kind="ExternalInput").ap()
    ones_d = nc.dram_tensor("ones_d", [1, 128], F32R, kind="ExternalInput").ap()
    id64_d = nc.dram_tensor("id64_d", [GPC, GPC], F32R, kind="ExternalInput").ap()
    out_stage = nc.dram_tensor("out_stage", [L, GPC, H], F32, kind="ExternalOutput").ap()
    xg_tm = nc.dram_tensor("xg_tm", [L, GPC, H3], F32R).ap()  # scratch

    # rank segments per 128-row tile: list of (tile, row0_in_tile, nrows,
    # rank, q0) covering [base_r, base_r + cap_r)
    segs_by_tile = [[] for _ in range(NT)]
    for r in range(GPC):
        q = 0
        while q < caps[r]:
            row = base[r] + q
            ti = row // 128
            take = min(caps[r] - q, (ti + 1) * 128 - row)
            segs_by_tile[ti].append((row - ti * 128, take, r, q))
            q += take

    with tile.TileContext(nc) as tc:
        with (
            tc.tile_pool(name="wpool", bufs=1) as wpool,
            tc.tile_pool(name="consts", bufs=1) as consts,
            tc.tile_pool(name="state", bufs=2) as state,
        ):
            ident = consts.tile([128, 128], F32)
            make_identity(nc, ident)
            ones_r = consts.tile([1, 128], F32R)
            nc.sync.dma_start(out=ones_r, in_=ones_d)
            biasT_s = consts.tile([128, KC], F32)
            nc.sync.dma_start(out=biasT_s, in_=biasT)
            pbias_s = consts.tile([1, H3], F32R)
            nc.sync.dma_start(out=pbias_s, in_=pbias)
            bhh_n_s = consts.tile([1, H], F32R)
            nc.sync.dma_start(out=bhh_n_s, in_=bhh_n)
            id64_s = consts.tile([GPC, GPC], F32R)
            nc.sync.dma_start(out=id64_s, in_=id64_d)

            # ---------------- Phase A: projection (compact layout) --------
            w_ih_s = wpool.tile([128, KC, H3], BF16, tag="w")
            for k in range(KC):
                nc.sync.dma_start(out=w_ih_s[:, k, :], in_=w_ihT[k])

            h0T_f = consts.tile([128, KC, GPC], F32)  # raw h0^T (f32)
            h0tmp = consts.tile([128, 1], F32)

            with (
                tc.tile_pool(name="pa", bufs=2, space="PSUM") as pa,
                tc.tile_pool(name="pt", bufs=2, space="PSUM") as pt,
                tc.tile_pool(name="aw", bufs=2) as aw,
            ):
                for ti in range(NT):
                    if not segs_by_tile[ti]:
                        continue
                    hp = aw.tile([128, H], F32, tag="hp")
                    nc.sync.dma_start(out=hp, in_=h_cmp[ti * 128:(ti + 1) * 128, :])
                    msgsT = aw.tile([128, KC, 128], BF16, tag="msgsT")
                    rawT = aw.tile([128, KC, 128], F32, tag="rawT")
                    for c in range(KC):
                        pst = pt.tile([128, 128], F32, tag="tp")
                        nc.tensor.transpose(pst, hp[:, c * 128:(c + 1) * 128], ident)
                        # drain pst fast (2 readers only) so PE transposes
                        # aren't stalled behind the per-rank h0 reduces
                        nc.vector.tensor_copy(rawT[:, c, :], pst)
                        nc.scalar.activation(
                            msgsT[:, c, :], pst, AF.Relu, bias=biasT_s[:, c:c + 1])
                        for (r0, nr, rank, q0) in segs_by_tile[ti]:
                            if q0 == 0:
                                nc.vector.tensor_reduce(
                                    out=h0T_f[:, c, rank:rank + 1],
                                    in_=rawT[:, c, r0:r0 + nr], axis=AX.X, op=OP.max)
                            else:
                                nc.vector.tensor_reduce(
                                    out=h0tmp, in_=rawT[:, c, r0:r0 + nr],
                                    axis=AX.X, op=OP.max)
                                nc.vector.tensor_tensor(
                                    h0T_f[:, c, rank:rank + 1],
                                    h0T_f[:, c, rank:rank + 1], h0tmp, op=OP.max)
                    for half in range(2):
                        pm = pa.tile([128, 1536], F32, tag="pm")
                        for b in range(3):
                            col0 = half * 1536 + b * 512
                            nc.tensor.matmul(
                                pm[:, b * 512:(b + 1) * 512], ones_r,
                                pbias_s[:, col0:col0 + 512], start=True, stop=False)
                            for k in range(KC):
                                nc.tensor.matmul(
                                    pm[:, b * 512:(b + 1) * 512], msgsT[:, k, :],
                                    w_ih_s[:, k, col0:col0 + 512],
                                    start=False, stop=(k == KC - 1))
                        xgs = aw.tile([128, 1536], F32R, tag="xgs")
                        if half == 0:
                            nc.scalar.copy(xgs, pm)
                        else:
                            nc.vector.tensor_copy(xgs, pm)
                        for (r0, nr, rank, q0) in segs_by_tile[ti]:
                            t0 = q0 + shift[rank]
                            nc.sync.dma_start(
                                out=xg_tm[t0:t0 + nr, rank,
                                          half * 1536:(half + 1) * 1536],
                                in_=xgs[r0:r0 + nr, :])

            # ---------------- Phase B: recurrence ----------------
            w_hh_s = wpool.tile([128, KC, H3], BF16, tag="w")
            for k in range(KC):
                nc.sync.dma_start(out=w_hh_s[:, k, :], in_=w_hhT[k])

            # initial state: hT (f32r) and h (natural layout)
            if htsplit:
                hT_lo = state.tile([128, KC // 2, GPC], BF16, tag="hTl")
                hT_hi = state.tile([128, KC // 2, GPC], BF16, tag="hTh")
                nc.scalar.copy(hT_lo, h0T_f[:, :KC // 2, :])
                nc.scalar.copy(hT_hi, h0T_f[:, KC // 2:, :])
                hT_pair = (hT_lo, hT_hi)
            else:
                hT = state.tile([128, KC, GPC], BF16, tag="hT")
                nc.scalar.copy(hT, h0T_f)
            h_s = state.tile([64, H], F32, tag="h")
            with tc.tile_pool(name="pi", bufs=2, space="PSUM") as pti:
                for c in range(KC):
                    pst = pti.tile([GPC, 128], F32, tag="tp")
                    nc.tensor.transpose(pst, h0T_f[:, c, :], ident)
                    nc.scalar.copy(h_s[:, c * 128:(c + 1) * 128], pst)

            with (
                tc.tile_pool(name="pb", bufs=1, space="PSUM") as pb,
                tc.tile_pool(name="ptr", bufs=2, space="PSUM") as ptrp,
                tc.tile_pool(name="gw", bufs=gwbufs) as gw,
                tc.tile_pool(name="sw", bufs=1) as sw,
                tc.tile_pool(name="xn", bufs=xnbufs) as xnp,
            ):
                for t in range(n_steps):
                    xg_s = xnp.tile([64, H3], F32R, tag="xg")
                    k_t = kreal[t]
                    if k_t > 0:
                        nc.sync.dma_start(out=xg_s[:k_t, :], in_=xg_tm[t, :k_t, :])
                    if k_t < GPC:
                        nc.sync.dma_start(out=xg_s[k_t:, :], in_=padgr[k_t:, :])

                    pr = pb.tile([64, H], F32, tag="pr")
                    pn = pb.tile([64, H], F32, tag="pn")
                    pz = pb.tile([64, H], F32, tag="pz")

                    r = gw.tile([64, H], F32, tag="r")
                    u = gw.tile([64, H], F32, tag="u")
                    n_g = gw.tile([64, H], F32, tag="n")
                    z = gw.tile([64, H], F32, tag="z")
                    d = sw.tile([64, H], F32, tag="d")
                    e = sw.tile([64, H], F32, tag="e")
                    h_new = state.tile([64, H], F32, tag="h")
                    if htsplit:
                        hTn_lo = state.tile([128, KC // 2, GPC], BF16, tag="hTl")
                        hTn_hi = state.tile([128, KC // 2, GPC], BF16, tag="hTh")
                    else:
                        hT_next = state.tile([128, KC, GPC], BF16, tag="hT")
                    ptr_t = ptrp.tile([128, 512], F32, tag="ptr")

                    # matmuls: per gate, inject xg/bhh per half (f32r,
                    # 512-wide) then 8 bf16 1024-wide W matmuls
                    for gate in ("r", "n", "z"):
                        tile_, wcol = {"r": (pr, 0), "z": (pz, 1024), "n": (pn, 2048)}[gate]
                        for hf in range(2):
                            sl = slice(hf * 512, hf * 512 + 512)
                            if gate == "n":
                                nc.tensor.matmul(tile_[:, sl], ones_r[:, :GPC],
                                                 bhh_n_s[:, sl], start=True, stop=False)
                            else:
                                nc.tensor.matmul(
                                    tile_[:, sl], id64_s,
                                    xg_s[:, wcol + hf * 512:wcol + hf * 512 + 512],
                                    start=True, stop=False)
                        for hf in range(2):
                            sl = slice(hf * 512, hf * 512 + 512)
                            for k in range(KC):
                                nc.tensor.matmul(
                                    tile_[:, sl], hT[:, k, :],
                                    w_hh_s[:, k, wcol + hf * 512:wcol + hf * 512 + 512],
                                    start=False, stop=(k == KC - 1))

                    for hf in range(2):
                        sl = slice(hf * 512, hf * 512 + 512)

                        # gate chain for this half
                        nc.scalar.activation(r[:, sl], pr[:, sl], AF.Sigmoid)
                        nc.vector.tensor_tensor(u[:, sl], r[:, sl], pn[:, sl], op=OP.mult)
                        nc.vector.tensor_tensor(
                            u[:, sl], u[:, sl],
                            xg_s.bitcast(F32)[:, 2048 + hf * 512:2048 + hf * 512 + 512],
                            op=OP.add)
                        nc.scalar.activation(n_g[:, sl], u[:, sl], AF.Tanh)
                        (nc.gpsimd if dsub == 'gp' else nc.vector).tensor_sub(d[:, sl], h_s[:, sl], n_g[:, sl])
                        for q in range(2):
                            qs = slice(hf * 512 + q * 256, hf * 512 + q * 256 + 256)
                            nc.scalar.activation(z[:, qs], pz[:, qs], AF.Sigmoid)
                            nc.vector.tensor_tensor(e[:, qs], z[:, qs], d[:, qs], op=OP.mult)
                            nc.vector.tensor_tensor(h_new[:, qs], n_g[:, qs], e[:, qs], op=OP.add)
                            if t < n_steps - 1:
                                c0 = 4 * hf + q * 2
                                for c in (c0, c0 + 1):
                                    nc.tensor.transpose(
                                        ptr_t[:, c * 64:(c + 1) * 64],
                                        h_new[:, c * 128:(c + 1) * 128],
                                        ident[:64, :64])
                                cp = nc.vector.tensor_copy if hf else nc.scalar.copy
                                if htsplit:
                                    dst = (hTn_lo if c0 < 4 else hTn_hi)[:, c0 % 4:c0 % 4 + 2, :]
                                else:
                                    dst = hT_next[:, c0:c0 + 2, :]
                                cp(dst, ptr_t[:, c0 * 64:(c0 + 2) * 64])

                    nout = max(k_t, 1)
                    nc.sync.dma_start(out=out_stage[t, :nout, :], in_=h_new[:nout, :])
                    if t < n_steps - 1:
                        if htsplit:
                            hT_pair = (hTn_lo, hTn_hi)
                        else:
                            hT = hT_next
                    h_s = h_new

    nc.compile()
    return nc, SP


def _get_programs(caps):
    key = tuple(caps)
    if key not in _PROG_CACHE:
        nf, SPf = _build_program("f", caps, int(caps[0]), dsub="gp")
        nb, SPb = _build_program("b", caps, L, dsub="gp")
        _PROG_CACHE[key] = (nf, nb, SPf)
    return _PROG_CACHE[key]


def _make_runner(nc, n_group, dev_offset):
    import jax
    from jax.sharding import Mesh, PartitionSpec
    from jax.experimental.shard_map import shard_map
    import concourse.mybir as mybir
    from concourse.bass2jax import (_bass_exec_p, install_neuronx_cc_hook,
                                    partition_id_tensor)

    install_neuronx_cc_hook()
    pname = nc.partition_id_tensor.name if nc.partition_id_tensor else None
    in_names, out_names, out_avals, zero_outs = [], [], [], []
    for alloc in nc.m.functions[0].allocations:
        if not isinstance(alloc, mybir.__dict__["MemoryLocationSet"]):
            continue
        name = alloc.memorylocations[0].name
        if alloc.kind == "ExternalInput":
            if name != pname:
                in_names.append(name)
        elif alloc.kind == "ExternalOutput":
            out_names.append(name)
            shape = tuple(alloc.tensor_shape)
            dtype = mybir.dt.np(alloc.dtype)
            out_avals.append(jax.core.ShapedArray(shape, dtype))
            zero_outs.append(np.zeros(shape, dtype))
    n_params = len(in_names)
    n_outs = len(out_avals)
    all_in = in_names + out_names + ([pname] if pname else [])

    def _body(*args):
        ops = list(args)
        if pname is not None:
            ops.append(partition_id_tensor())
        outs = _bass_exec_p.bind(
            *ops, out_avals=tuple(out_avals), in_names=tuple(all_in),
            out_names=tuple(out_names), lowering_input_output_aliases=(),
            sim_require_finite=True, sim_require_nnan=True, nc=nc)
        return tuple(outs)

    devices = jax.devices()[dev_offset:dev_offset + n_group]
    mesh = Mesh(np.asarray(devices), ("core",))
    jf = jax.jit(shard_map(_body, mesh=mesh,
                           in_specs=(PartitionSpec("core"),) * (n_params + n_outs),
                           out_specs=(PartitionSpec("core"),) * n_outs,
                           check_rep=False), keep_unused=True)
    sh = jax.sharding.NamedSharding(mesh, PartitionSpec("core"))

    def run(in_maps):
        import jax as _jax
        concat = [np.concatenate([np.asarray(in_maps[c][nm]) for c in range(n_group)], axis=0)
                  for nm in in_names]
        concat += [np.concatenate([z] * n_group, axis=0) for z in zero_outs]
        dev_in = [_jax.device_put(a, sh) for a in concat]
        res = jf(*dev_in)
        return res, out_names

    return run


def kernel(h_atom, bias, w_ih_f, w_hh_f, b_ih_f, b_hh_f,
           w_ih_b, w_hh_b, b_ih_b, b_hh_b, batch, num_graphs, pad_len):
    import jax

    h_atom = np.asarray(h_atom, dtype=np.float32)
    batch_np = np.asarray(batch).astype(np.int64)
    n_atoms = h_atom.shape[0]

    counts = np.bincount(batch_np, minlength=G).astype(np.int64)
    start = np.concatenate([[0], np.cumsum(counts)[:-1]])
    pos = np.arange(n_atoms) - start[batch_np]

    # sort graphs by count desc, deal 4-way -> per-core rank lists with a
    # shared capacity profile
    order = np.argsort(-counts, kind="stable")
    ranks = [order[4 * np.arange(GPC) + c] for c in range(4)]  # graph ids per core mod
    caps = []
    for i in range(GPC):
        m = max(counts[order[4 * i + c]] for c in range(4))
        caps.append(int(max(8, ((m + 7) // 8) * 8)))
    caps[0] = min(caps[0], L)
    caps = [min(c, L) for c in caps]

    nf, nb, SP = _get_programs(caps)
    base = np.concatenate([[0], np.cumsum(caps)]).astype(int)

    def prep_core(core):
        d = "f" if core < 4 else "b"
        glist = ranks[core % 4]
        w_ih = np.asarray(w_ih_f if d == "f" else w_ih_b, dtype=np.float32)
        w_hh = np.asarray(w_hh_f if d == "f" else w_hh_b, dtype=np.float32)
        b_ih = np.asarray(b_ih_f if d == "f" else b_ih_b, dtype=np.float32)
        b_hh = np.asarray(b_hh_f if d == "f" else b_hh_b, dtype=np.float32)

        hc = np.full((SP, H), PAD_VAL, dtype=np.float32)
        for i, g in enumerate(glist):
            rows = h_atom[start[g]:start[g] + min(int(counts[g]), caps[i])]
            # bwd: reversed atoms end-aligned within the cap range, so slot q
            # lands at step q + L - cap_i, i.e. atom (orig pos l) at step L-1-l
            if d == "b":
                rows = rows[::-1]
                off = caps[i] - len(rows)
            else:
                off = 0
            hc[base[i] + off:base[i] + off + len(rows)] = rows

        pb = np.empty((1, H3), dtype=np.float32)
        pb[0, :2048] = b_ih[:2048] + b_hh[:2048]
        pb[0, 2048:] = b_ih[2048:]
        return {
            "h_cmp": hc,
            "w_ihT": np.ascontiguousarray(w_ih.T).reshape(KC, 128, H3)
                .astype(_bf16),
            "w_hhT": np.ascontiguousarray(w_hh.T).reshape(KC, 128, H3)
                .astype(_bf16),
            "biasT": np.ascontiguousarray(
                np.asarray(bias, dtype=np.float32).reshape(KC, 128).T),
            "pbias": pb,
            "padgr": np.tile(pb, (GPC, 1)),
            "bhh_n": b_hh[2048:].reshape(1, H).astype(np.float32),
            "ones_d": np.ones((1, 128), dtype=np.float32),
            "id64_d": np.eye(GPC, dtype=np.float32),
        }

    in_maps = [prep_core(c) for c in range(N_CORES)]
    global _LAST_IN_MAPS
    _LAST_IN_MAPS = in_maps

    rkey = ("runners",) + tuple(caps)
    if rkey not in _PROG_CACHE:
        _PROG_CACHE[rkey] = (_make_runner(nf, 4, 0), _make_runner(nb, 4, 4))
    run_f, run_b = _PROG_CACHE[rkey]

    res_f, names_f = run_f(in_maps[:4])
    res_b, names_b = run_b(in_maps[4:])
    jax.block_until_ready(res_f)
    jax.block_until_ready(res_b)

    stage_f = np.asarray(res_f[names_f.index("out_stage")])  # [4*L, GPC, H]
    stage_b = np.asarray(res_b[names_b.index("out_stage")])

    out = np.empty((n_atoms, 2 * H), dtype=np.float32)
    for core in range(4):
        glist = ranks[core]
        inv = np.empty(G, dtype=np.int64)
        inv[glist] = np.arange(GPC)
        sel = np.isin(batch_np, glist)
        gi = inv[batch_np[sel]]
        p = np.minimum(pos[sel], L - 1)
        out[sel, :H] = stage_f[core * L + p, gi]
        out[sel, H:] = stage_b[core * L + (L - 1 - p), gi]
    return out

